# revision 25
# baseline (speedup 1.0000x reference)
"""Distributed Trainium2 kernel for ANEMultiHeadAttention.

Problem: B=2, C=1024, S=2048, H=16, D=64.
  x: (B, C, 1, S);  q = Wq x + bq; k = Wk x; v = Wv x + bv
  per-head attention (softmax over keys), out = Wo o + bo.

Sharding (8 cores): core i handles batch b = i // 4 and head-group
hg = i % 4 (4 heads = 256 channels). Q/K/V column-parallel, Wo
row-parallel; host sums the 4 partial outputs per batch.
The v-bias contributes Wo @ bv (softmax rows sum to 1) and is folded
into a host-side constant along with bo.

Per-core device algorithm (all matmuls bf16, f32 PSUM):
  - q = WqT_s^T @ x (+bq), k = WkT_s^T @ x; vT = x^T @ WvT_s stored
    (128, head, 65) per key-tile with a ones column (the PV matmul then
    also accumulates softmax denominators).
  - attention in q-windows of 512, head pairs row-packed: BOTH heads'
    scoresT land in ONE (128, 1024) PSUM tile (different banks), one
    exp instruction (ACT, scale 1/8) covers both heads -> the exp
    stream on the Scalar engine runs back-to-back (it is the kernel's
    critical path); PV: o_aug += vT_aug^T @ expT per head.
  - QKV / vT / out-projection work is emission-interleaved into the
    attention loops so the PE stays dense while ACT works.
  - normalize (recip + partition_broadcast + mul) runs off the hot
    path after a fast PSUM->SBUF evac.
"""

import sys

for p in ("/opt/trn_rl_repo",):
    if p not in sys.path:
        sys.path.insert(0, p)

from contextlib import ExitStack

import ml_dtypes
import numpy as np

import concourse.bass as bass
import concourse.mybir as mybir
import concourse.tile as tile
from concourse import bacc
from concourse.bass_utils import run_bass_kernel_spmd

# Problem shape (hardcoded per contest rules)
B, C, S, H = 2, 1024, 2048, 16
D = C // H  # 64
N_CORES = 8
HG = 4  # head groups
HPG = H // HG  # heads per group = 4
CPG = HPG * D  # channels per group = 256
P = 128
NK = C // P  # 8 contraction tiles over C
NST = S // P  # 16 key tiles
QW = 512  # q window per head
NQW = S // QW  # 4
WIN = 1024  # qk-projection unit width

F32 = mybir.dt.float32
BF16 = mybir.dt.bfloat16
EXP = mybir.ActivationFunctionType.Exp

_CACHED_NC = None


def build_nc():
    nc = bacc.Bacc("TRN2", target_bir_lowering=False, debug=False)

    x_d = nc.dram_tensor("x", (P, NK, S), BF16, kind="ExternalInput")
    wq_d = nc.dram_tensor("wqT", (P, NK, CPG), BF16, kind="ExternalInput")
    wk_d = nc.dram_tensor("wkT", (P, NK, CPG), BF16, kind="ExternalInput")
    wv_d = nc.dram_tensor("wvT", (P, NK, CPG), BF16, kind="ExternalInput")
    wo_d = nc.dram_tensor("woT", (P, 2, C), BF16, kind="ExternalInput")
    bq_d = nc.dram_tensor("bq", (P, 2), F32, kind="ExternalInput")
    out_d = nc.dram_tensor("out", (P, NK, S), F32, kind="ExternalOutput")

    with tile.TileContext(nc) as tc, ExitStack() as ctx:
        const = ctx.enter_context(tc.tile_pool(name="const", bufs=1))
        work = ctx.enter_context(tc.tile_pool(name="work", bufs=1))
        expp = ctx.enter_context(tc.tile_pool(name="expp", bufs=8))
        onp = ctx.enter_context(tc.tile_pool(name="onp", bufs=6))
        outp = ctx.enter_context(tc.tile_pool(name="outp", bufs=4))
        smal = ctx.enter_context(tc.tile_pool(name="smal", bufs=6))
        psc = ctx.enter_context(tc.tile_pool(name="psc", bufs=2, space="PSUM"))
        ppv = ctx.enter_context(tc.tile_pool(name="ppv", bufs=4, space="PSUM"))

        # ---- weights first (small, needed immediately), then x first-half,
        # then x second-half, so the pair-0 projections start earliest ----
        wq_sb = const.tile([P, NK, CPG], BF16, tag="wq")
        nc.sync.dma_start(wq_sb[:], wq_d[:])
        wk_sb = const.tile([P, NK, CPG], BF16, tag="wk")
        nc.sync.dma_start(wk_sb[:], wk_d[:])
        wv_sb = const.tile([P, NK, CPG], BF16, tag="wv")
        nc.sync.dma_start(wv_sb[:], wv_d[:])
        wo_sb = const.tile([P, 2, C], BF16, tag="wo")
        nc.sync.dma_start(wo_sb[:], wo_d[:])
        bq_sb = const.tile([P, 2], F32, tag="bq")
        nc.sync.dma_start(bq_sb[:], bq_d[:])
        xh = [[None, None] for _ in range(NK)]
        for half in range(2):
            for kt in range(NK):
                t = const.tile([P, WIN], BF16, tag=f"x{kt}h{half}", name=f"x{kt}h{half}")
                nc.sync.dma_start(
                    t[:], x_d[:, kt, half * WIN : (half + 1) * WIN]
                )
                xh[kt][half] = t

        # activations, one tile per (pair, window/chunk) for fine deps
        def wtile(nm):
            return work.tile([P, WIN], BF16, tag=nm, name=nm)

        k_t = [[wtile(f"k{p_}c{c}") for c in range(2)] for p_ in range(2)]
        q_t = [[wtile(f"q{p_}w{w}") for w in range(2)] for p_ in range(2)]
        o_t = [[wtile(f"o{p_}w{w}") for w in range(2)] for p_ in range(2)]
        vt = [
            work.tile([P, HPG, D + 1], BF16, tag=f"vt{st}", name=f"vt{st}")
            for st in range(NST)
        ]

        def qk_mms(ps, w_sb, pair, c, kt):
            for ch in range(2):
                nc.tensor.matmul(
                    ps[:, ch * 512 : (ch + 1) * 512],
                    w_sb[:, kt, pair * P : (pair + 1) * P],
                    xh[kt][c][:, ch * 512 : (ch + 1) * 512],
                    start=(kt == 0),
                    stop=(kt == NK - 1),
                )

        def qk_evac(ps, dst, pair, bias):
            if bias:
                nc.vector.tensor_scalar_add(
                    dst[:], ps[:], bq_sb[:, pair : pair + 1]
                )
            else:
                nc.vector.tensor_copy(dst[:], ps[:])

        def qk_unit(w_sb, dst, pair, c, bias):
            ps = psc.tile([P, WIN], F32, tag="psc", name="ps_qk")
            for kt in range(NK):
                qk_mms(ps, w_sb, pair, c, kt)
            qk_evac(ps, dst, pair, bias)

        def vt_mm(ps, st, kt):
            half, off = divmod(st * P, WIN)
            nc.tensor.matmul(
                ps[:, :CPG],
                xh[kt][half][:, off : off + P],
                wv_sb[:, kt, :],
                start=(kt == 0),
                stop=(kt == NK - 1),
            )

        def vt_evac(ps, st):
            nc.vector.tensor_copy(
                vt[st][:, :, 0:D],
                ps[:, :CPG].rearrange("p (h d) -> p h d", h=HPG),
            )

        def vt_unit(st):
            nc.vector.memset(vt[st][:], 1.0)
            ps = ppv.tile([P, QW], F32, tag="ppv", name="ps_vt")
            for kt in range(NK):
                vt_mm(ps, st, kt)
            vt_evac(ps, st)

        def attention(pair, qw, inject=None):
            inject = inject or {}
            w, half = divmod(qw, 2)
            qs = slice(half * QW, (half + 1) * QW)
            oa = ppv.tile([P, QW], F32, tag="ppv", name="oa")
            ob = ppv.tile([P, QW], F32, tag="ppv", name="ob")

            def pv(prev):
                pkt, pe = prev
                for acc, hoff, cs in (
                    (oa, 0, slice(0, QW)),
                    (ob, 1, slice(QW, 2 * QW)),
                ):
                    nc.tensor.matmul(
                        acc[0 : D + 1, :],
                        vt[pkt][:, 2 * pair + hoff, :],
                        pe[:, cs],
                        start=(pkt == 0),
                        stop=(pkt == NST - 1),
                    )

            prev = None
            for kt in range(NST):
                s = psc.tile([P, WIN], F32, tag="psc", name="s")
                c, j = divmod(kt, NK)
                for rlo, rhi, cs, tpos in (
                    (0, D, slice(0, QW), (0, 0)),
                    (D, P, slice(QW, 2 * QW), (64, 0)),
                ):
                    nc.tensor.matmul(
                        s[:, cs],
                        k_t[pair][c][rlo:rhi, j * P : (j + 1) * P],
                        q_t[pair][w][rlo:rhi, qs],
                        tile_position=tpos,
                    )
                e = expp.tile([P, WIN], BF16, tag="exp", name="e")
                nc.scalar.activation(e[:], s[:], EXP, scale=float(D) ** -0.5)
                if prev is not None:
                    pv(prev)
                prev = (kt, e)
                for f in inject.get(kt, ()):
                    f()
            pv(prev)

            # fast evac to SBUF first (frees both PSUM slots before the slow
            # reciprocal runs), then normalize off the hot path
            ous = []
            for acc in (oa, ob):
                ou = onp.tile([D + 1, QW], F32, tag="ou", name="ou")
                nc.vector.tensor_copy(ou[:], acc[0 : D + 1, :])
                ous.append(ou)
            # one reciprocal for both heads (cost scales with free size, not
            # partitions); rows live at partitions 0/32 (legal slice bases),
            # and head B's result bounces to partition 0 so
            # partition_broadcast reads from its base partition
            den = smal.tile([33, QW], F32, tag="den", name="den")
            nc.vector.tensor_copy(den[0:1, :], ous[0][D : D + 1, :])
            nc.vector.tensor_copy(den[32:33, :], ous[1][D : D + 1, :])
            rc = smal.tile([33, QW], F32, tag="rc", name="rc")
            # one instruction covers both rows (partitions 1..31 are unread)
            nc.vector.reciprocal(rc[:], den[:])
            rc_b = smal.tile([1, QW], F32, tag="rc_b", name="rc_b")
            nc.vector.tensor_copy(rc_b[:], rc[32:33, :])
            for head, ou in enumerate(ous):
                rcb = smal.tile([D, QW], F32, tag="rcb", name="rcb")
                nc.gpsimd.partition_broadcast(
                    rcb[:], rc[0:1, :] if head == 0 else rc_b[:]
                )
                nc.vector.tensor_mul(
                    o_t[pair][w][head * D : (head + 1) * D, qs],
                    ou[0:D, :],
                    rcb[:],
                )

        def outproj_unit(qw, m):
            w, half = divmod(qw, 2)
            cs = slice(half * QW, (half + 1) * QW)
            ps = ppv.tile([P, QW], F32, tag="ppv", name="ps_out")
            for kt in range(2):
                nc.tensor.matmul(
                    ps[:],
                    wo_sb[:, kt, m * P : (m + 1) * P],
                    o_t[kt][w][:, cs],
                    start=(kt == 0),
                    stop=(kt == 1),
                )
            ot = outp.tile([P, QW], F32, tag="ot", name="ot")
            nc.vector.tensor_copy(ot[:], ps[:])
            nc.sync.dma_start(out_d[:, m, qw * QW : (qw + 1) * QW], ot[:])

        # ---- emission schedule ----
        # Head phase: k/q for pair 0 window 0, interleaved per contraction
        # tile so the (first-half) x DMAs pipeline straight into the PE.
        ps_k = psc.tile([P, WIN], F32, tag="psc", name="ps_k")
        ps_q = psc.tile([P, WIN], F32, tag="psc", name="ps_q")
        # HAM warm-up: ~4us of matmuls on the already-arrived weights while
        # the x DMAs land; results are discarded by the first start=True
        wk_flat = wk_sb[:].rearrange("p a b -> p (a b)")
        for i in range(10):
            nc.tensor.matmul(
                ps_k[:, 0:512] if i % 2 == 0 else ps_q[:, 0:512],
                wk_sb[:, 0, 0:P],
                wk_flat[:, 0:512],
            )
        for kt in range(NK):
            qk_mms(ps_k, wk_sb, 0, 0, kt)
            qk_mms(ps_q, wq_sb, 0, 0, kt)
        qk_evac(ps_k, k_t[0][0], 0, False)
        qk_evac(ps_q, q_t[0][0], 0, True)

        def U(f, *a):
            return lambda: f(*a)

        VT = lambda st: U(vt_unit, st)  # noqa: E731
        QK = lambda wsb, dst, p_, c_, b_: U(qk_unit, wsb, dst, p_, c_, b_)  # noqa: E731
        OP = lambda qw_, m_: U(outproj_unit, qw_, m_)  # noqa: E731

        attention(
            0,
            0,
            {
                0: (VT(0), VT(1)),
                1: (VT(2), VT(3)),
                3: (VT(4), VT(5)),
                5: (VT(6), VT(7), QK(wk_sb, k_t[0][1], 0, 1, False)),
                8: (VT(8), VT(9)),
                10: (VT(10), VT(11)),
                12: (VT(12), VT(13)),
                14: (VT(14), VT(15)),
            },
        )
        attention(
            0,
            1,
            {
                2: (QK(wk_sb, k_t[1][0], 1, 0, False),),
                8: (QK(wq_sb, q_t[1][0], 1, 0, True),),
                14: (QK(wk_sb, k_t[1][1], 1, 1, False),),
            },
        )
        attention(1, 0, {})
        attention(
            1,
            1,
            {
                2: (QK(wq_sb, q_t[0][1], 0, 1, True),),
                11: (OP(0, 0), OP(0, 1), OP(0, 2)),
                13: (OP(0, 3), OP(0, 4), OP(0, 5)),
                15: (OP(0, 6), OP(0, 7)),
            },
        )
        attention(
            0,
            2,
            {
                2: (QK(wq_sb, q_t[1][1], 1, 1, True),),
                11: (OP(1, 0), OP(1, 1), OP(1, 2)),
                13: (OP(1, 3), OP(1, 4), OP(1, 5)),
                15: (OP(1, 6), OP(1, 7)),
            },
        )
        attention(1, 2, {})
        attention(
            0,
            3,
            {
                11: (OP(2, 0), OP(2, 1), OP(2, 2)),
                13: (OP(2, 3), OP(2, 4), OP(2, 5)),
                15: (OP(2, 6), OP(2, 7)),
            },
        )
        attention(1, 3, {})
        for m in range(NK):
            outproj_unit(3, m)

    nc.compile()
    return nc


def _shard_inputs(hidden_states, Wq, bq, Wk, Wv, bv, Wo, bo):
    bf = ml_dtypes.bfloat16
    in_maps = []
    for core in range(N_CORES):
        b, hg = divmod(core, HG)
        x = hidden_states[b, :, 0, :]  # (C, S) f32
        cs = slice(hg * CPG, (hg + 1) * CPG)
        wqT = Wq[cs, :].T.reshape(NK, P, CPG).transpose(1, 0, 2)
        wkT = Wk[cs, :].T.reshape(NK, P, CPG).transpose(1, 0, 2)
        wvT = Wv[cs, :].T.reshape(NK, P, CPG).transpose(1, 0, 2)
        woT = Wo[:, cs].T.reshape(2, P, C).transpose(1, 0, 2)
        in_maps.append(
            {
                "x": np.ascontiguousarray(
                    x.reshape(NK, P, S).transpose(1, 0, 2)
                ).astype(bf),
                "wqT": np.ascontiguousarray(wqT).astype(bf),
                "wkT": np.ascontiguousarray(wkT).astype(bf),
                "wvT": np.ascontiguousarray(wvT).astype(bf),
                "woT": np.ascontiguousarray(woT).astype(bf),
                "bq": np.ascontiguousarray(
                    bq[cs].reshape(2, P).T
                ).astype(np.float32),
            }
        )
    return in_maps


def get_nc():
    global _CACHED_NC
    if _CACHED_NC is None:
        _CACHED_NC = build_nc()
    return _CACHED_NC


def run(hidden_states, Wq, bq, Wk, Wv, bv, Wo, bo, trace=False, **kw):
    nc = get_nc()
    in_maps = _shard_inputs(hidden_states, Wq, bq, Wk, Wv, bv, Wo, bo)
    res = run_bass_kernel_spmd(
        nc, in_maps, core_ids=list(range(N_CORES)), trace=trace, **kw
    )
    # unshard: sum partials per batch, add host-side constant bias
    bias_vec = (Wo.astype(np.float64) @ bv.astype(np.float64)).astype(
        np.float32
    ) + bo
    out = np.zeros((B, C, 1, S), dtype=np.float32)
    for core in range(N_CORES):
        b = core // HG
        part = np.asarray(res.results[core]["out"], dtype=np.float32)
        out[b, :, 0, :] += part.transpose(1, 0, 2).reshape(C, S)
    out[:, :, 0, :] += bias_vec[None, :, None]
    return out, res


def _run_subprocess(inputs):
    """Retry path for transient device failures: a fresh interpreter gets a
    fresh PJRT/device state."""
    import os
    import pickle
    import subprocess
    import tempfile

    kdir = os.path.dirname(os.path.abspath(__file__))
    with tempfile.TemporaryDirectory() as td:
        inp = os.path.join(td, "in.pkl")
        outp = os.path.join(td, "out.pkl")
        with open(inp, "wb") as f:
            pickle.dump(inputs, f)
        code = (
            "import pickle, sys; sys.path.insert(0, %r); import kernel;\n"
            "inputs = pickle.load(open(%r, 'rb'));\n"
            "out, _ = kernel.run(**inputs);\n"
            "pickle.dump(out, open(%r, 'wb'))\n" % (kdir, inp, outp)
        )
        subprocess.run([sys.executable, "-c", code], check=True, timeout=1500)
        with open(outp, "rb") as f:
            return pickle.load(f)


def kernel(**inputs):
    try:
        out, _ = run(**inputs)
        return out
    except Exception:
        pass
    # transient NRT_EXEC_UNIT_UNRECOVERABLE happens occasionally; retry in
    # fresh subprocesses (fresh device handles)
    last = None
    for _ in range(3):
        try:
            return _run_subprocess(inputs)
        except Exception as e:  # noqa: PERF203
            last = e
    raise last
